# revision 4
# baseline (speedup 1.0000x reference)
"""Trainium2 Bass kernel for nn_BasicBlock (binary-activation conv block).

Reference forward (per element):
    act  = sign(x + b0)                      # {-1, 0, +1}
    bw   = scale_c * sign(w),  scale_c = mean|w| over (ci,kh,kw)
    raw  = conv3x3(act, sign(w))             # exact small integers
    y    = (scale*raw - mu) * rsqrt(var + eps) * gamma + beta + x + b1
    out  = prelu(y, alpha) + b2
with BN stats (mu, var) over the FULL batch (sync-BN across cores).

Strategy (8 NeuronCores, batch-sharded 4 imgs/core):
  - act/weights are +-1 -> bf16 matmuls with fp32 PSUM accumulation are EXACT.
  - conv = 9 shifted matmuls (K=64, M=64) packed 4-at-a-time into the 128x128
    PE array quadrants via tile_position (2 psum tiles x 2 quadrants each).
  - two-pass conv: pass 1 -> bn_stats only (psum discarded); tiny AllReduce of
    per-channel (sum, sumsq); pass 2 recomputes conv and fuses the epilogue:
        psum = A*raw + x         (one DVE scalar_tensor_tensor)
        out  = Prelu(psum + B)   (one ACT pass, per-channel alpha)
    where A = gamma*scale*rsqrt(var+eps), B = beta + b1 - mu*A.
  - all weight prep (sign, scale, transpose) happens on-device.

kernel(**inputs) takes FULL inputs, shards, runs SPMD on cores 0-7, gathers.
"""
import numpy as np
from contextlib import ExitStack

from concourse import bacc, mybir, tile
from concourse.bass_utils import run_bass_kernel_spmd

# ---------------- problem constants (hardcoded per spec) ----------------
N_CORES = 8
IMGS = 4          # images per core
C = 64            # channels
H = W = 112
HP = WP = 114     # zero-padded act dims
BN_EPS = 1e-5
NG = 32 * H * W   # global BN count per channel

f32 = mybir.dt.float32
bf16 = mybir.dt.bfloat16

RPC = 8           # output rows per psum chunk (2 psum banks)
NCHUNK = H // RPC  # 14 row-chunks per image-slot
PSUM_BUFS = 3
SIGN_BLK = 28     # rows per sign block


def build_program(with_b2: bool, trace_friendly: bool = False):
    nc = bacc.Bacc("TRN2", target_bir_lowering=False, debug=False,
                   num_devices=N_CORES)

    x_d = nc.dram_tensor("x", [IMGS, C, H, W], f32, kind="ExternalInput")
    b0_d = nc.dram_tensor("b0", [1, C, 1, 1], f32, kind="ExternalInput")
    w_d = nc.dram_tensor("w", [C, C, 3, 3], f32, kind="ExternalInput")
    gamma_d = nc.dram_tensor("gamma", [C], f32, kind="ExternalInput")
    beta_d = nc.dram_tensor("beta", [C], f32, kind="ExternalInput")
    b1_d = nc.dram_tensor("b1", [1, C, 1, 1], f32, kind="ExternalInput")
    alpha_d = nc.dram_tensor("alpha", [C], f32, kind="ExternalInput")
    b2_d = nc.dram_tensor("b2", [1, C, 1, 1], f32, kind="ExternalInput")
    ident_d = nc.dram_tensor("ident", [64, 64], f32, kind="ExternalInput")
    out_d = nc.dram_tensor("out", [IMGS, C, H, W], f32, kind="ExternalOutput")

    AF = mybir.ActivationFunctionType
    OP = mybir.AluOpType

    with tile.TileContext(nc) as tc, ExitStack() as ctx:
        pool = ctx.enter_context(tc.tile_pool(name="sbuf", bufs=1))
        outp = ctx.enter_context(tc.tile_pool(name="outp", bufs=4))
        stgp = ctx.enter_context(tc.tile_pool(name="stage", bufs=2))
        psum = ctx.enter_context(
            tc.tile_pool(name="psum", bufs=PSUM_BUFS, space="PSUM"))
        wps = ctx.enter_context(tc.tile_pool(name="wps", bufs=1, space="PSUM"))
        dram = ctx.enter_context(tc.tile_pool(name="dram", bufs=1, space="DRAM"))

        # ---------------- small params ----------------
        # par[64, k]: b0 g beta b1 alpha b2
        par = pool.tile([64, 6], f32)
        nc.sync.dma_start(par[:, 0:1], b0_d.ap().rearrange("a c e f -> (a c) (e f)"))
        nc.sync.dma_start(par[:, 1:2], gamma_d.ap().rearrange("c -> c ()"))
        nc.sync.dma_start(par[:, 2:3], beta_d.ap().rearrange("c -> c ()"))
        nc.sync.dma_start(par[:, 3:4], b1_d.ap().rearrange("a c e f -> (a c) (e f)"))
        nc.sync.dma_start(par[:, 4:5], alpha_d.ap().rearrange("c -> c ()"))
        nc.sync.dma_start(par[:, 5:6], b2_d.ap().rearrange("a c e f -> (a c) (e f)"))
        # replicated to both partition halves: cols = b0, alpha, b2
        rep = pool.tile([128, 3], f32)
        nc.vector.tensor_copy(rep[0:64, 0:1], par[:, 0:1])
        nc.vector.tensor_copy(rep[0:64, 1:2], par[:, 4:5])
        nc.vector.tensor_copy(rep[0:64, 2:3], par[:, 5:6])
        nc.sync.dma_start(rep[64:128, :], rep[0:64, :])
        b0_ap = rep[:, 0:1]
        alpha_ap = rep[:, 1:2]

        # ---------------- weight prep (on device) ----------------
        w_sb = pool.tile([64, 576], f32)
        nc.sync.dma_start(w_sb[:], w_d.ap().rearrange("o i kh kw -> o (i kh kw)"))
        scale_sb = pool.tile([64, 1], f32)
        nc.vector.tensor_reduce(scale_sb[:], w_sb[:], axis=mybir.AxisListType.X,
                                op=OP.add, apply_absolute_value=True)
        nc.vector.tensor_scalar(scale_sb[:], scale_sb[:], 1.0 / 576.0, None,
                                op0=OP.mult)
        # sgn(w) = 2*(w>0)-1  (exact, matches reference incl. w==0 -> -1)
        sgnw = pool.tile([64, 576], f32)
        nc.vector.tensor_scalar(sgnw[:], w_sb[:], 0.0, None, op0=OP.is_gt)
        nc.vector.tensor_scalar(sgnw[:], sgnw[:], 2.0, -1.0, op0=OP.mult, op1=OP.add)
        sgnw_bf = pool.tile([64, 576], bf16)
        nc.vector.tensor_copy(sgnw_bf[:], sgnw[:])
        # identity for PE transpose
        ident_f = pool.tile([64, 64], f32)
        nc.sync.dma_start(ident_f[:], ident_d[:])
        ident_bf = pool.tile([64, 64], bf16)
        nc.vector.tensor_copy(ident_bf[:], ident_f[:])
        # per-tap transposed weights [ci, co], replicated on both halves
        w_taps = pool.tile([128, 9, 64], bf16)
        sgn_view = sgnw_bf[:].rearrange("o (i t) -> o t i", t=9)
        for t in range(9):
            wtp = wps.tile([64, 64], bf16)
            nc.tensor.transpose(wtp[:], sgn_view[:, t, :], ident_bf[:])
            nc.vector.tensor_copy(w_taps[0:64, t, :], wtp[:])
        nc.sync.dma_start(w_taps[64:128, :, :], w_taps[0:64, :, :])

        # ---------------- x load + sign ----------------
        # x_sb slots match PSUM/output layout:
        #   slot0 = (img0 | img1), slot1 = (img2 | img3)
        x_sb = pool.tile([128, 2, H, W], f32)
        x_v = x_d.ap().rearrange("i c h w -> (i c) h w")
        for s in range(2):
            for blk in range(2):
                r0, r1 = blk * 56, (blk + 1) * 56
                src = x_v[128 * s:128 * (s + 1), r0:r1, :]
                nc.sync.dma_start(x_sb[:, s, r0:r1, :], src)

        # act slots (conv-input layout): h0 = (img0, img3), h1 = (img1, img2)
        act = pool.tile([128, 2, HP, WP], bf16)
        # zero borders: rows 0,113 and cols 0,113
        nc.gpsimd.memset(act[:, :, 0:114:113, :], 0.0)
        nc.gpsimd.memset(act[:, :, :, 0:114:113], 0.0)
        nblk = H // SIGN_BLK
        for b in range(nblk):
            r0, r1 = b * SIGN_BLK, (b + 1) * SIGN_BLK
            # slot0 (img0|img1): direct, same partitions
            nc.scalar.activation(act[:, 0, 1 + r0:1 + r1, 1:113],
                                 x_sb[:, 0, r0:r1, :], AF.Sign, bias=b0_ap)
            # slot1 (img2|img3): sign into stage, then swap halves via DMA
            stg = stgp.tile([128, SIGN_BLK, W], bf16)
            nc.scalar.activation(stg[:], x_sb[:, 1, r0:r1, :], AF.Sign, bias=b0_ap)
            nc.sync.dma_start(act[64:128, 1, 1 + r0:1 + r1, 1:113], stg[0:64, :, :])
            nc.sync.dma_start(act[0:64, 1, 1 + r0:1 + r1, 1:113], stg[64:128, :, :])

        # quadrant plan per slot: (act_half_base, tile_position, psum_base)
        QUADS = {
            0: [(0, (0, 0), 0), (64, (64, 64), 64)],     # img0 -> G0, img1 -> G64
            1: [(64, (64, 0), 0), (0, (0, 64), 64)],     # img2 -> G0, img3 -> G64
        }

        def conv_into(p_tile, s, r):
            """Accumulate the 9-tap binary conv for (slot s, row-chunk r).

            p_tile is [128, 2, 512]: one PSUM bank per 4-row half-chunk
            (cols 0:448 used, 448:512 pad) so each matmul stays in-bank."""
            for half in range(2):
                orow = r * RPC + half * 4
                dst = p_tile[:, half, 0:448].rearrange("p (r c) -> p r c", r=4)
                for t in range(9):
                    ky, kx = divmod(t, 3)
                    for (ab, tp, pb) in QUADS[s]:
                        rhs = act[ab:ab + 64, s, orow + ky:orow + ky + 4,
                                  kx:kx + 112]
                        nc.tensor.matmul(
                            dst[pb:pb + 64], w_taps[ab:ab + 64, t, :], rhs,
                            start=(t == 0), stop=(t == 8), tile_position=tp)

        # ---------------- phase 1: conv -> batch stats ----------------
        bnst = pool.tile([128, 2 * NCHUNK * 2, 6], f32)
        for s in range(2):
            for r in range(NCHUNK):
                p1 = psum.tile([128, 2, 512], f32, tag="cv")
                conv_into(p1, s, r)
                k = (s * NCHUNK + r) * 2
                nc.vector.bn_stats(bnst[:, k, :], p1[:, 0, 0:448])
                nc.vector.bn_stats(bnst[:, k + 1, :], p1[:, 1, 0:448])

        # ---------------- sync-BN: allreduce (sum, sumsq) ----------------
        mv = pool.tile([128, 2], f32)
        nc.vector.bn_aggr(mv[:], bnst[:])
        NL = float(IMGS // 2 * H * W)  # elements per partition (2 imgs)
        ssq = pool.tile([128, 2], f32)
        nc.vector.tensor_scalar(ssq[:, 0:1], mv[:, 0:1], NL, None, op0=OP.mult)
        # sumsq = (mean^2 + var) * NL
        nc.vector.scalar_tensor_tensor(ssq[:, 1:2], mv[:, 0:1], mv[:, 0:1],
                                       mv[:, 1:2], op0=OP.mult, op1=OP.add)
        nc.vector.tensor_scalar(ssq[:, 1:2], ssq[:, 1:2], NL, None, op0=OP.mult)

        ar_in = dram.tile([128, 2], f32)
        ar_out = dram.tile([128, 2], f32)
        nc.sync.dma_start(ar_in[:], ssq[:])
        nc.gpsimd.collective_compute(
            "AllReduce", OP.add, ins=[ar_in.opt()], outs=[ar_out.opt()],
            replica_groups=[list(range(N_CORES))])
        g_sb = pool.tile([128, 2], f32)
        nc.sync.dma_start(g_sb[:], ar_out[:])
        # combine partition halves -> per-channel totals on partitions 0-63
        hswap = pool.tile([64, 2], f32)
        nc.sync.dma_start(hswap[:], g_sb[64:128, :])
        tot = pool.tile([64, 2], f32)
        nc.vector.tensor_tensor(tot[:], g_sb[0:64, :], hswap[:], op=OP.add)

        # ---------------- A, B computation (partitions 0-63) ----------------
        sc2 = pool.tile([64, 1], f32)   # scale^2
        nc.vector.tensor_tensor(sc2[:], scale_sb[:], scale_sb[:], op=OP.mult)
        mean_g = pool.tile([64, 1], f32)
        nc.vector.tensor_scalar(mean_g[:], tot[:, 0:1], 1.0 / NG, None, op0=OP.mult)
        ex2 = pool.tile([64, 1], f32)
        nc.vector.tensor_scalar(ex2[:], tot[:, 1:2], 1.0 / NG, None, op0=OP.mult)
        m2 = pool.tile([64, 1], f32)
        nc.vector.tensor_tensor(m2[:], mean_g[:], mean_g[:], op=OP.mult)
        var_r = pool.tile([64, 1], f32)  # raw-conv variance = ex2 - mean^2
        nc.vector.tensor_tensor(var_r[:], ex2[:], m2[:], op=OP.subtract)
        vpe = pool.tile([64, 1], f32)   # scale^2 * var + eps
        nc.vector.tensor_tensor(vpe[:], var_r[:], sc2[:], op=OP.mult)
        nc.vector.tensor_scalar(vpe[:], vpe[:], BN_EPS, None, op0=OP.add)
        # rsqrt via ACT sqrt + DVE reciprocal + one Newton step
        sq = pool.tile([64, 1], f32)
        nc.scalar.activation(sq[:], vpe[:], AF.Sqrt)
        r0_t = pool.tile([64, 1], f32)
        nc.vector.reciprocal(r0_t[:], sq[:])
        rr = pool.tile([64, 1], f32)
        nc.vector.tensor_tensor(rr[:], r0_t[:], r0_t[:], op=OP.mult)
        nc.vector.tensor_tensor(rr[:], rr[:], vpe[:], op=OP.mult)
        nc.vector.tensor_scalar(rr[:], rr[:], -0.5, 1.5, op0=OP.mult, op1=OP.add)
        rsq = pool.tile([64, 1], f32)
        nc.vector.tensor_tensor(rsq[:], r0_t[:], rr[:], op=OP.mult)
        # A = rsq * scale * gamma ; B = beta + b1 - mean*A
        ab = pool.tile([128, 2], f32)
        nc.vector.tensor_tensor(ab[0:64, 0:1], rsq[:], scale_sb[:], op=OP.mult)
        nc.vector.tensor_tensor(ab[0:64, 0:1], ab[0:64, 0:1], par[:, 1:2], op=OP.mult)
        mA = pool.tile([64, 1], f32)
        nc.vector.tensor_tensor(mA[:], mean_g[:], ab[0:64, 0:1], op=OP.mult)
        nc.vector.tensor_tensor(ab[0:64, 1:2], par[:, 2:3], par[:, 3:4], op=OP.add)
        nc.vector.tensor_tensor(ab[0:64, 1:2], ab[0:64, 1:2], mA[:], op=OP.subtract)
        nc.sync.dma_start(ab[64:128, :], ab[0:64, :])
        A_ap = ab[:, 0:1]
        B_ap = ab[:, 1:2]

        # ---------------- phase 2: conv -> fused epilogue -> out ----------------
        out_v = out_d.ap().rearrange("i c h w -> (i c) h w")
        for s in range(2):
            for r in range(NCHUNK):
                p2 = psum.tile([128, 2, 512], f32, tag="cv")
                conv_into(p2, s, r)
                # psum = A * raw + x   (per bank half)
                for half in range(2):
                    xr = x_sb[:, s, r * RPC + half * 4:r * RPC + (half + 1) * 4, :]
                    nc.vector.scalar_tensor_tensor(
                        p2[:, half, 0:448], p2[:, half, 0:448], A_ap,
                        xr.rearrange("p r c -> p (r c)"),
                        op0=OP.mult, op1=OP.add)
                ot = outp.tile([128, RPC, W], f32, tag="ot")
                nc.scalar.activation(
                    ot[:].rearrange("p (h r) c -> p h (r c)", h=2),
                    p2[:, :, 0:448], AF.Prelu, bias=B_ap,
                    scale=1.0, alpha=alpha_ap)
                if with_b2:
                    nc.vector.tensor_scalar(ot[:], ot[:], rep[:, 2:3], None,
                                            op0=OP.add)
                dst = out_v[128 * s:128 * (s + 1), r * RPC:(r + 1) * RPC, :]
                nc.sync.dma_start(dst, ot[:])

    nc.compile()
    return nc


_CACHE = {}


def _get_program(with_b2: bool):
    if with_b2 not in _CACHE:
        _CACHE[with_b2] = build_program(with_b2)
    return _CACHE[with_b2]


def run_sharded(inputs: dict, trace: bool = False, tmpdir=None):
    """Shard, run on 8 cores, gather. Returns (out, BassKernelResults)."""
    x = np.ascontiguousarray(np.asarray(inputs["x"], dtype=np.float32))
    w = np.ascontiguousarray(np.asarray(inputs["w"], dtype=np.float32))
    b0 = np.ascontiguousarray(np.asarray(inputs["b0"], dtype=np.float32))
    gamma = np.ascontiguousarray(np.asarray(inputs["gamma"], dtype=np.float32))
    beta = np.ascontiguousarray(np.asarray(inputs["beta"], dtype=np.float32))
    b1 = np.ascontiguousarray(np.asarray(inputs["b1"], dtype=np.float32))
    alpha = np.ascontiguousarray(np.asarray(inputs["alpha"], dtype=np.float32))
    b2 = np.ascontiguousarray(np.asarray(inputs["b2"], dtype=np.float32))
    with_b2 = bool(np.any(b2 != 0.0))
    nc = _get_program(with_b2)

    ident = np.eye(64, dtype=np.float32)
    in_maps = []
    for k in range(N_CORES):
        in_maps.append({
            "x": np.ascontiguousarray(x[IMGS * k:IMGS * (k + 1)]),
            "w": w, "b0": b0, "gamma": gamma, "beta": beta, "b1": b1,
            "alpha": alpha, "b2": b2, "ident": ident,
        })
    res = run_bass_kernel_spmd(nc, in_maps, list(range(N_CORES)),
                               trace=trace, tmpdir=tmpdir)
    out = np.concatenate([res.results[k]["out"] for k in range(N_CORES)], axis=0)
    return out, res


def kernel(**inputs) -> np.ndarray:
    out, _ = run_sharded(inputs, trace=False)
    return out
